# revision 2
# baseline (speedup 1.0000x reference)
"""MoE (top-2 of 8 experts, swiglu MLP) on 8 Trainium2 NeuronCores.

Strategy: expert parallelism — core e owns expert e's weights.
 - Host: router (fp64 softmax/top-2), gather each expert's tokens,
   pre-tile weights, cast everything to bf16.
 - Device (per core, SPMD one NEFF): single chunk of C=2048 token
   columns, 4 blocks of 512.  Stage A is h-major over all 8 PSUM banks
   (gate+up per h-tile), silu on ACT, mul on DVE into bf16 hT.
   Stage B is token-major: y[c, h] = hT.T @ down^T — weights stream as
   the moving operand, so stage B needs no DMA-in at all.
 - Host: combine — scale rows by gating weight, scatter-add into the
   full output.  Tokens beyond the C=2048 capacity of an expert are
   computed on host (~0.6% of tokens for the target input).

Shapes: T=8192 tokens, H=2048, F=1408, E=8, K=2.
"""

import numpy as np

T, H, E, K, F = 8192, 2048, 8, 2, 1408
C = 2048          # token capacity per expert column-space
NB = 4            # column blocks of 512
BW = 512
NH = 16           # h-blocks (H / 128)
NF = 11           # f-blocks (F / 128)
N_CORES = 8

_compiled = None


def _build():
    from contextlib import ExitStack

    import concourse.mybir as mybir
    import concourse.tile as tile
    from concourse import bacc

    f32 = mybir.dt.float32
    bf16 = mybir.dt.bfloat16

    nc = bacc.Bacc("TRN2", target_bir_lowering=False, debug=False, num_devices=N_CORES)
    # xt[p, hb, c] = x^T[hb*128+p, c] — partition-major so multi-h-block
    # slices are per-partition contiguous (1 MB DMAs instead of 512 KB)
    xt = nc.dram_tensor("xt", [128, NH, C], bf16, kind="ExternalInput").ap()
    gu = nc.dram_tensor("gu", [2, NF, 128, 2048], bf16, kind="ExternalInput").ap()
    dw = nc.dram_tensor("dw", [NF, 128, 2048], bf16, kind="ExternalInput").ap()
    yt = nc.dram_tensor("yt", [C, H], f32, kind="ExternalOutput").ap()

    with tile.TileContext(nc) as tc:
        with ExitStack() as ctx:
            pool_xt1 = ctx.enter_context(tc.tile_pool(name="xt1", bufs=2))
            pool_xt2 = ctx.enter_context(tc.tile_pool(name="xt2", bufs=7))
            pool_gu = ctx.enter_context(tc.tile_pool(name="gu", bufs=4))
            pool_dw = ctx.enter_context(tc.tile_pool(name="dw", bufs=NF))
            pool_h = ctx.enter_context(tc.tile_pool(name="h", bufs=NF))
            pool_sil = ctx.enter_context(tc.tile_pool(name="sil", bufs=4))
            pool_out = ctx.enter_context(tc.tile_pool(name="out", bufs=4))
            ps = ctx.enter_context(tc.tile_pool(name="ps", bufs=8, space="PSUM"))

            # f=0 weights first, split into pieces so the PE can start as
            # soon as the first piece + first xt tile land (scalar queue;
            # xt goes on sync so the two HWDGE rings run in parallel)
            gut0 = pool_gu.tile([128, 2048], bf16, tag="gut", name="gut")
            uut0 = pool_gu.tile([128, 2048], bf16, tag="uut", name="uut")
            for q in range(4):
                nc.scalar.dma_start(
                    gut0[:, q * 512 : (q + 1) * 512], gu[0, 0, :, q * 512 : (q + 1) * 512]
                )
            for q in range(4):
                nc.scalar.dma_start(
                    uut0[:, q * 512 : (q + 1) * 512], gu[1, 0, :, q * 512 : (q + 1) * 512]
                )

            # token activations, H on partitions.  First two h-blocks come
            # as singles (PE can start early), the rest as 1 MB pairs.
            # h0-9 ride the sync ring; h10-15 ride scalar after the f=0
            # weights, so the two HWDGE rings drain in parallel.
            xparts = [(0, 1), (1, 1)] + [(h, 2) for h in range(2, NH, 2)]
            xcol = [None] * NH  # h -> (tile, col_base)
            for start, n in xparts:
                pool = pool_xt1 if n == 1 else pool_xt2
                xtile = pool.tile([128, n, C], bf16, tag=f"xt{n}")
                eng = nc.sync if start < 10 else nc.scalar
                eng.dma_start(xtile[:], xt[:, start : start + n, :])
                for j in range(n):
                    xcol[start + j] = (xtile, j)

            dwts = [None] * NF

            # stage A: hT[f, c] = silu(gT) * uT,  gT = gate_w.T @ x.T
            hts = []
            for f in range(NF):
                ht = pool_h.tile([128, C], bf16, tag="ht")
                hts.append(ht)
                if f == 0:
                    gut, uut = gut0, uut0
                else:
                    gut = pool_gu.tile([128, 2048], bf16, tag="gut", name="gut")
                    uut = pool_gu.tile([128, 2048], bf16, tag="uut", name="uut")
                    nc.scalar.dma_start(gut[:], gu[0, f, :, :])
                    nc.scalar.dma_start(uut[:], gu[1, f, :, :])
                # down weights trickled in mid-stage-A (needed only in B)
                if 2 <= f < 2 + NF // 2:
                    for j in (2 * (f - 2), 2 * (f - 2) + 1):
                        dwt = pool_dw.tile([128, 2048], bf16, tag="dwt")
                        nc.scalar.dma_start(dwt[:], dw[j, :, :])
                        dwts[j] = dwt
                elif f == 2 + NF // 2:
                    dwt = pool_dw.tile([128, 2048], bf16, tag="dwt")
                    nc.scalar.dma_start(dwt[:], dw[NF - 1, :, :])
                    dwts[NF - 1] = dwt

                pgs = [ps.tile([128, BW], f32, tag="ps", name=f"pg{b}") for b in range(NB)]
                pus = [ps.tile([128, BW], f32, tag="ps", name=f"pu{b}") for b in range(NB)]
                for h in range(NH):
                    xtile, xj = xcol[h]
                    for b in range(NB):
                        nc.tensor.matmul(
                            pgs[b][:],
                            gut[:, h * 128 : (h + 1) * 128],
                            xtile[:, xj, b * BW : (b + 1) * BW],
                            start=(h == 0),
                            stop=(h == NH - 1),
                        )
                    for b in range(NB):
                        nc.tensor.matmul(
                            pus[b][:],
                            uut[:, h * 128 : (h + 1) * 128],
                            xtile[:, xj, b * BW : (b + 1) * BW],
                            start=(h == 0),
                            stop=(h == NH - 1),
                        )
                for b in range(NB):
                    sil = pool_sil.tile([128, BW], f32, tag="sil")
                    nc.scalar.activation(
                        sil[:], pgs[b][:], mybir.ActivationFunctionType.Silu
                    )
                    nc.vector.tensor_mul(
                        hts[f][:, b * BW : (b + 1) * BW], sil[:], pus[b][:]
                    )

            # stage B: y[c, h] = hT.T @ down^T — hT slices are the
            # stationary operand (one LDWEIGHTS per 4 matmuls), down^T
            # streams from SBUF; no DMA-in needed.
            for cb in range(NH):
                pos = [ps.tile([128, BW], f32, tag="ps", name=f"po{q}") for q in range(NB)]
                for f in range(NF):
                    for q in range(NB):
                        nc.tensor.matmul(
                            pos[q][:],
                            hts[f][:, cb * 128 : (cb + 1) * 128],
                            dwts[f][:, q * BW : (q + 1) * BW],
                            start=(f == 0),
                            stop=(f == NF - 1),
                        )
                for q in range(NB):
                    ot = pool_out.tile([128, BW], f32, tag="ot")
                    nc.vector.tensor_copy(ot[:], pos[q][:])
                    eng = nc.scalar if q % 2 == 0 else nc.sync
                    eng.dma_start(
                        yt[cb * 128 : (cb + 1) * 128, q * BW : (q + 1) * BW], ot[:]
                    )
    nc.compile()
    return nc


def _get_compiled():
    global _compiled
    if _compiled is None:
        _compiled = _build()
    return _compiled


def _route(x, router_w):
    """fp64 router: returns per-expert (indices, gating weights)."""
    logits = x.astype(np.float64) @ router_w.astype(np.float64).T
    logits -= logits.max(axis=-1, keepdims=True)
    p = np.exp(logits)
    p /= p.sum(axis=-1, keepdims=True)
    top2 = np.argsort(-p, axis=-1)[:, :K]
    pv = np.take_along_axis(p, top2, axis=-1)
    wts = pv / (pv.sum(axis=-1, keepdims=True) + 1e-20)
    idxs, gws = [], []
    for e in range(E):
        tok, pos = np.nonzero(top2 == e)
        idxs.append(tok.astype(np.int64))
        gws.append(wts[tok, pos].astype(np.float32))
    return idxs, gws


def _tile_gu(wT, bf16):
    # gu[f_blk, k, hb*128+m] = wT[hb*128+k, f_blk*128+m]
    return (
        wT.reshape(16, 128, 11, 128)
        .transpose(2, 1, 0, 3)
        .reshape(11, 128, 2048)
        .astype(bf16)
    )


def _swiglu_host(xg, gate, up, down):
    g = xg @ gate.T
    u = xg @ up.T
    h = (g / (1.0 + np.exp(-g))) * u
    return h @ down.T


def kernel(hidden_states, router_w, gate_w, up_w, down_w):
    import ml_dtypes
    from concourse import bass_utils

    bf16 = ml_dtypes.bfloat16

    x = np.ascontiguousarray(hidden_states.reshape(-1, H).astype(np.float32))
    idxs, gws = _route(x, router_w)
    xbf = x.astype(bf16)

    in_maps = []
    spill = []  # (expert, token_indices, gate_weights) handled on host
    for e in range(E):
        idx = idxs[e]
        if len(idx) > C:
            spill.append((e, idx[C:], gws[e][C:]))
            idx = idx[:C]
        xtf = np.zeros((H, C), dtype=bf16)
        xtf[:, : len(idx)] = xbf[idx].T
        xt = np.ascontiguousarray(xtf.reshape(NH, 128, C).transpose(1, 0, 2))
        guw = np.stack(
            [
                _tile_gu(gate_w[e].T.astype(np.float32), bf16),
                _tile_gu(up_w[e].T.astype(np.float32), bf16),
            ]
        )
        dwt = np.ascontiguousarray(
            down_w[e].T.reshape(NF, 128, 2048)
        ).astype(bf16)
        in_maps.append({"xt": xt, "gu": guw, "dw": dwt})

    global _last_in_maps
    _last_in_maps = in_maps
    nc = _get_compiled()
    res = bass_utils.run_bass_kernel_spmd(nc, in_maps, core_ids=list(range(N_CORES)))

    out = np.zeros((T, H), dtype=np.float32)
    for e in range(E):
        # token indices are unique within one expert (a token's two experts
        # are distinct), so fancy-index += is an exact scatter-add
        idx = idxs[e][:C]
        w = gws[e][:C]
        y = res.results[e]["yt"][: len(idx), :]
        out[idx] += w[:, None] * y
    for e, idx, w in spill:
        y = _swiglu_host(x[idx], gate_w[e], up_w[e], down_w[e]).astype(np.float32)
        out[idx] += w[:, None] * y
    return out.reshape(hidden_states.shape).astype(np.float32)


# revision 3
# speedup vs baseline: 1.0092x; 1.0092x over previous
"""MoE (top-2 of 8 experts, swiglu MLP) on 8 Trainium2 NeuronCores.

Strategy: expert parallelism — core e owns expert e's weights.
 - Host: router (fp64 softmax/top-2), gather each expert's tokens,
   pre-tile weights, cast everything to bf16.
 - Device (per core, SPMD one NEFF): single chunk of C=2048 token
   columns, 4 blocks of 512.  Stage A is h-major over all 8 PSUM banks
   (gate+up per h-tile), silu on ACT, mul on DVE into bf16 hT.
   Stage B is token-major: y[c, h] = hT.T @ down^T — weights stream as
   the moving operand, so stage B needs no DMA-in at all.
 - Host: combine — scale rows by gating weight, scatter-add into the
   full output.  Tokens beyond the C=2048 capacity of an expert are
   computed on host (~0.6% of tokens for the target input).

Shapes: T=8192 tokens, H=2048, F=1408, E=8, K=2.
"""

import numpy as np

T, H, E, K, F = 8192, 2048, 8, 2, 1408
C = 2048          # token capacity per expert column-space
NB = 4            # column blocks of 512
BW = 512
NH = 16           # h-blocks (H / 128)
NF = 11           # f-blocks (F / 128)
N_CORES = 8

_compiled = None


def _build():
    from contextlib import ExitStack

    import concourse.mybir as mybir
    import concourse.tile as tile
    from concourse import bacc

    f32 = mybir.dt.float32
    bf16 = mybir.dt.bfloat16

    nc = bacc.Bacc("TRN2", target_bir_lowering=False, debug=False, num_devices=N_CORES)
    # xt[p, hb, c] = x^T[hb*128+p, c] — partition-major so multi-h-block
    # slices are per-partition contiguous (1 MB DMAs instead of 512 KB)
    xt = nc.dram_tensor("xt", [128, NH, C], bf16, kind="ExternalInput").ap()
    gu = nc.dram_tensor("gu", [2, NF, 128, 2048], bf16, kind="ExternalInput").ap()
    dw = nc.dram_tensor("dw", [NF, 128, 2048], bf16, kind="ExternalInput").ap()
    yt = nc.dram_tensor("yt", [C, H], f32, kind="ExternalOutput").ap()

    with tile.TileContext(nc) as tc:
        with ExitStack() as ctx:
            pool_xt1 = ctx.enter_context(tc.tile_pool(name="xt1", bufs=2))
            pool_xt2 = ctx.enter_context(tc.tile_pool(name="xt2", bufs=7))
            pool_gu = ctx.enter_context(tc.tile_pool(name="gu", bufs=4))
            pool_dw = ctx.enter_context(tc.tile_pool(name="dw", bufs=NF))
            pool_h = ctx.enter_context(tc.tile_pool(name="h", bufs=NF))
            pool_sil = ctx.enter_context(tc.tile_pool(name="sil", bufs=4))
            pool_out = ctx.enter_context(tc.tile_pool(name="out", bufs=4))
            ps = ctx.enter_context(tc.tile_pool(name="ps", bufs=8, space="PSUM"))

            # f=0 weights first, gate/up pieces interleaved (tiny leading
            # pieces) so the U matmuls of h=0 aren't blocked behind all of
            # gut0; scalar ring — xt h0-9 rides sync in parallel.
            gut0 = pool_gu.tile([128, 2048], bf16, tag="gut", name="gut")
            uut0 = pool_gu.tile([128, 2048], bf16, tag="uut", name="uut")
            nc.scalar.dma_start(gut0[:, 0:128], gu[0, 0, :, 0:128])
            nc.scalar.dma_start(uut0[:, 0:128], gu[1, 0, :, 0:128])

            # token activations, H on partitions.  First two h-blocks come
            # as singles (PE can start early), the rest as 1 MB pairs.
            # h0-9 ride the sync ring; h10-15 ride scalar, with the f=1
            # weights woven between the pairs so nothing stalls f=1.
            xcol = [None] * NH  # h -> (tile, sub-index)

            def xpart(start, n, eng):
                pool = pool_xt1 if n == 1 else pool_xt2
                xtile = pool.tile([128, n, C], bf16, tag=f"xt{n}")
                eng.dma_start(xtile[:], xt[:, start : start + n, :])
                for j in range(n):
                    xcol[start + j] = (xtile, j)

            xpart(0, 1, nc.scalar)
            for o, e in ((128, 1024), (1024, 2048)):
                nc.scalar.dma_start(gut0[:, o:e], gu[0, 0, :, o:e])
                nc.scalar.dma_start(uut0[:, o:e], gu[1, 0, :, o:e])
            xpart(1, 1, nc.sync)
            for h in range(2, 10, 2):
                xpart(h, 2, nc.sync)
            xpart(10, 2, nc.scalar)
            gut1 = pool_gu.tile([128, 2048], bf16, tag="gut", name="gut")
            uut1 = pool_gu.tile([128, 2048], bf16, tag="uut", name="uut")
            for o, e in ((0, 1024), (1024, 2048)):
                nc.scalar.dma_start(gut1[:, o:e], gu[0, 1, :, o:e])
                nc.scalar.dma_start(uut1[:, o:e], gu[1, 1, :, o:e])
            xpart(12, 2, nc.scalar)
            xpart(14, 2, nc.scalar)

            dwts = [None] * NF

            # stage A: hT[f, c] = silu(gT) * uT,  gT = gate_w.T @ x.T
            hts = []
            guts = [gut0, gut1]
            uuts = [uut0, uut1]
            for f in range(NF):
                ht = pool_h.tile([128, C], bf16, tag="ht")
                hts.append(ht)
                gut, uut = guts[f], uuts[f]
                # prefetch next f's weights, emitted at the top of the body
                # so the scalar ring isn't blocked behind this f's silus
                if 1 <= f < NF - 1:
                    gnext = pool_gu.tile([128, 2048], bf16, tag="gut", name="gut")
                    unext = pool_gu.tile([128, 2048], bf16, tag="uut", name="uut")
                    nc.scalar.dma_start(gnext[:], gu[0, f + 1, :, :])
                    nc.scalar.dma_start(unext[:], gu[1, f + 1, :, :])
                    guts.append(gnext)
                    uuts.append(unext)
                # down weights trickled in mid-stage-A (needed only in B)
                if 2 <= f < 2 + NF // 2:
                    for j in (2 * (f - 2), 2 * (f - 2) + 1):
                        dwt = pool_dw.tile([128, 2048], bf16, tag="dwt")
                        nc.scalar.dma_start(dwt[:], dw[j, :, :])
                        dwts[j] = dwt
                elif f == 2 + NF // 2:
                    dwt = pool_dw.tile([128, 2048], bf16, tag="dwt")
                    nc.scalar.dma_start(dwt[:], dw[NF - 1, :, :])
                    dwts[NF - 1] = dwt

                pgs = [ps.tile([128, BW], f32, tag="ps", name=f"pg{b}") for b in range(NB)]
                pus = [ps.tile([128, BW], f32, tag="ps", name=f"pu{b}") for b in range(NB)]
                for h in range(NH):
                    xtile, xj = xcol[h]
                    for b in range(NB):
                        nc.tensor.matmul(
                            pgs[b][:],
                            gut[:, h * 128 : (h + 1) * 128],
                            xtile[:, xj, b * BW : (b + 1) * BW],
                            start=(h == 0),
                            stop=(h == NH - 1),
                        )
                    for b in range(NB):
                        nc.tensor.matmul(
                            pus[b][:],
                            uut[:, h * 128 : (h + 1) * 128],
                            xtile[:, xj, b * BW : (b + 1) * BW],
                            start=(h == 0),
                            stop=(h == NH - 1),
                        )
                for b in range(NB):
                    sil = pool_sil.tile([128, BW], f32, tag="sil")
                    nc.scalar.activation(
                        sil[:], pgs[b][:], mybir.ActivationFunctionType.Silu
                    )
                    nc.vector.tensor_mul(
                        hts[f][:, b * BW : (b + 1) * BW], sil[:], pus[b][:]
                    )

            # stage B: y[c, h] = hT.T @ down^T — hT slices are the
            # stationary operand (one LDWEIGHTS per 4 matmuls), down^T
            # streams from SBUF; no DMA-in needed.
            for cb in range(NH):
                pos = [ps.tile([128, BW], f32, tag="ps", name=f"po{q}") for q in range(NB)]
                for f in range(NF):
                    for q in range(NB):
                        nc.tensor.matmul(
                            pos[q][:],
                            hts[f][:, cb * 128 : (cb + 1) * 128],
                            dwts[f][:, q * BW : (q + 1) * BW],
                            start=(f == 0),
                            stop=(f == NF - 1),
                        )
                for q in range(NB):
                    ot = pool_out.tile([128, BW], f32, tag="ot")
                    nc.vector.tensor_copy(ot[:], pos[q][:])
                    eng = nc.scalar if q % 2 == 0 else nc.sync
                    eng.dma_start(
                        yt[cb * 128 : (cb + 1) * 128, q * BW : (q + 1) * BW], ot[:]
                    )
    nc.compile()
    return nc


def _get_compiled():
    global _compiled
    if _compiled is None:
        _compiled = _build()
    return _compiled


def _route(x, router_w):
    """fp64 router: returns per-expert (indices, gating weights)."""
    logits = x.astype(np.float64) @ router_w.astype(np.float64).T
    logits -= logits.max(axis=-1, keepdims=True)
    p = np.exp(logits)
    p /= p.sum(axis=-1, keepdims=True)
    top2 = np.argsort(-p, axis=-1)[:, :K]
    pv = np.take_along_axis(p, top2, axis=-1)
    wts = pv / (pv.sum(axis=-1, keepdims=True) + 1e-20)
    idxs, gws = [], []
    for e in range(E):
        tok, pos = np.nonzero(top2 == e)
        idxs.append(tok.astype(np.int64))
        gws.append(wts[tok, pos].astype(np.float32))
    return idxs, gws


def _tile_gu(wT, bf16):
    # gu[f_blk, k, hb*128+m] = wT[hb*128+k, f_blk*128+m]
    return (
        wT.reshape(16, 128, 11, 128)
        .transpose(2, 1, 0, 3)
        .reshape(11, 128, 2048)
        .astype(bf16)
    )


def _swiglu_host(xg, gate, up, down):
    g = xg @ gate.T
    u = xg @ up.T
    h = (g / (1.0 + np.exp(-g))) * u
    return h @ down.T


def kernel(hidden_states, router_w, gate_w, up_w, down_w):
    import ml_dtypes
    from concourse import bass_utils

    bf16 = ml_dtypes.bfloat16

    x = np.ascontiguousarray(hidden_states.reshape(-1, H).astype(np.float32))
    idxs, gws = _route(x, router_w)
    xbf = x.astype(bf16)

    in_maps = []
    spill = []  # (expert, token_indices, gate_weights) handled on host
    for e in range(E):
        idx = idxs[e]
        if len(idx) > C:
            spill.append((e, idx[C:], gws[e][C:]))
            idx = idx[:C]
        xtf = np.zeros((H, C), dtype=bf16)
        xtf[:, : len(idx)] = xbf[idx].T
        xt = np.ascontiguousarray(xtf.reshape(NH, 128, C).transpose(1, 0, 2))
        guw = np.stack(
            [
                _tile_gu(gate_w[e].T.astype(np.float32), bf16),
                _tile_gu(up_w[e].T.astype(np.float32), bf16),
            ]
        )
        dwt = np.ascontiguousarray(
            down_w[e].T.reshape(NF, 128, 2048)
        ).astype(bf16)
        in_maps.append({"xt": xt, "gu": guw, "dw": dwt})

    global _last_in_maps
    _last_in_maps = in_maps
    nc = _get_compiled()
    res = bass_utils.run_bass_kernel_spmd(nc, in_maps, core_ids=list(range(N_CORES)))

    out = np.zeros((T, H), dtype=np.float32)
    for e in range(E):
        # token indices are unique within one expert (a token's two experts
        # are distinct), so fancy-index += is an exact scatter-add
        idx = idxs[e][:C]
        w = gws[e][:C]
        y = res.results[e]["yt"][: len(idx), :]
        out[idx] += w[:, None] * y
    for e, idx, w in spill:
        y = _swiglu_host(x[idx], gate_w[e], up_w[e], down_w[e]).astype(np.float32)
        out[idx] += w[:, None] * y
    return out.reshape(hidden_states.shape).astype(np.float32)
